# revision 2
# baseline (speedup 1.0000x reference)
"""MAGNN metapath-instance message-passing layer on 8 Trainium2 NeuronCores.

Strategy:
  - Partition destination nodes into 8 contiguous ranges (one per core); every
    edge (metapath instance) is routed to the core that owns its destination, so
    the per-destination softmax and weighted aggregation stay device-local.
  - Per core, edges are sorted by destination and packed into B buckets of 128
    consecutive node ids, padded to T tiles of 128 edges each (uniform program
    across cores = SPMD).
  - Per 128-edge tile, on device:
      * indirect-DMA gather of the 4 node-feature rows per edge  [128, 256] f32
      * PE transpose -> matmul against a host-built linear map CM [256, 72]
        producing [hidden(64) | attention-logits(8)] per edge (the rotation,
        mean-over-path and both attention dot products are one linear map)
      * leaky-relu + exp (max-free softmax numerator; logits are O(10) so
        exp() is safe in fp32)
      * z = [p (x) hidden | p]  [128, 520], one-hot dst matrix [128, 128]
      * PE matmul one-hot^T @ z accumulated over the bucket's tiles =
        per-node numerators W and denominators S
  - bucket finalize: ft = W / (S + eps), plain DMA to the per-core output rows.
  - Host concatenates the 8 per-core outputs. No collectives needed.
"""

import sys

if "/opt/trn_rl_repo" not in sys.path:
    sys.path.insert(0, "/opt/trn_rl_repo")

import numpy as np

import concourse.bacc as bacc
import concourse.mybir as mybir
from concourse.bass import IndirectOffsetOnAxis
from concourse.bass_utils import run_bass_kernel_spmd
from concourse.tile import TileContext

# problem constants (fixed by the harness contract)
N_CORES = 8
N = 50000
E = 200000
L = 4
OD = 64
NH = 8
D2 = OD // 2
ETYPES = (0, 2, 4)
ALPHA = 0.01

PT = 128          # edges per tile == nodes per bucket
NPC = 6272        # nodes per core; 8 * 6272 = 50176 >= 50000
B = NPC // PT     # 49 buckets per core
EPS = 1e-30

_PROGRAM_CACHE: dict = {}


def _linear_maps(rvec, attn1_w, attn2):
    """CM [2*128, 72]: edata_flat[e, 256] @ CM = [hidden(64) | a1+a2 logits(8)].

    Mirrors the reference RotatE0 composition in fp32.
    """
    rvec = np.asarray(rvec, np.float32)
    w1 = np.asarray(attn1_w, np.float32)            # [NH, OD]
    w2 = np.asarray(attn2, np.float32).reshape(NH, OD)

    rv = (rvec / np.linalg.norm(rvec, axis=2, keepdims=True)).astype(np.float32)
    conj = rv * np.array([1.0, -1.0], dtype=np.float32)
    rv2 = np.stack([rv, conj], axis=1).reshape(-1, D2, 2)

    f = [None] * L
    f[L - 1] = np.stack([np.ones(D2, np.float32), np.zeros(D2, np.float32)], -1)
    for i in range(L - 2, -1, -1):
        r = rv2[ETYPES[i]]
        re = f[i + 1][:, 0] * r[:, 0] - f[i + 1][:, 1] * r[:, 1]
        im = f[i + 1][:, 0] * r[:, 1] + f[i + 1][:, 1] * r[:, 0]
        f[i] = np.stack([re, im], -1).astype(np.float32)
    frv = np.stack(f)                                # [L, D2, 2]

    M = np.zeros((L * OD, OD), np.float32)
    for l in range(L):
        fr = frv[l, :, 0]
        fi = frv[l, :, 1]
        k = np.arange(D2)
        M[l * OD + 2 * k, 2 * k] = fr / L
        M[l * OD + 2 * k + 1, 2 * k] = -fi / L
        M[l * OD + 2 * k, 2 * k + 1] = fi / L
        M[l * OD + 2 * k + 1, 2 * k + 1] = fr / L

    C = M @ w2.T                                     # [256, NH]
    C[(L - 1) * OD:, :] += w1.T                      # center @ attn1_w.T term
    return np.concatenate([M, C], axis=1).astype(np.float32)   # [256, 72]


def _prep_core(mp, core, T, npc=NPC, nbuckets=B):
    """Per-core index streams.

    Returns gidx [128, nbuckets*T*4] int32 (gather node ids, 0 for padding)
    and dstloc [128, nbuckets*T] float32 (dst - bucket_base, 1000.0 for pad).
    """
    dst = mp[:, -1]
    lo, hi = core * npc, (core + 1) * npc
    sel = np.nonzero((dst >= lo) & (dst < hi))[0]
    d = dst[sel] - lo
    order = np.argsort(d, kind="stable")
    sel = sel[order]
    d = d[order]

    bucket = d >> 7                                   # //128
    starts = np.searchsorted(bucket, np.arange(nbuckets))
    pos = np.arange(len(sel)) - starts[bucket]        # rank within bucket
    cap = T * PT

    slot_edge = np.full(nbuckets * cap, -1, np.int64)
    slot_edge[bucket * cap + pos] = sel               # assumes pos < cap (T chosen so)
    slot_edge = slot_edge.reshape(nbuckets * T, PT)   # [tile, p]

    valid = slot_edge >= 0
    safe = np.where(valid, slot_edge, 0)

    g = mp[safe, :].astype(np.int32)                  # [tile, p, 4]
    g[~valid] = 0
    gidx = np.ascontiguousarray(g.transpose(1, 0, 2).reshape(PT, -1))

    base = (np.arange(nbuckets * T) // T * PT)[:, None]  # bucket base per tile
    dl = np.where(valid, dst[safe] - lo - base, 1000.0).astype(np.float32)
    dstloc = np.ascontiguousarray(dl.T)               # [128, ntiles]
    return gidx, dstloc


def _build(T, nfeat=N, npc=NPC, nbuckets=B):
    """Build + compile the per-core Bass program (same program for all cores)."""
    NT = nbuckets * T
    f32 = mybir.dt.float32

    nc = bacc.Bacc("TRN2", target_bir_lowering=False, debug=False,
                   num_devices=N_CORES)

    feat_d = nc.dram_tensor("feat", [nfeat, OD], f32, kind="ExternalInput")
    cm_d = nc.dram_tensor("cm", [128, 144], f32, kind="ExternalInput")
    consts_d = nc.dram_tensor("consts", [128, 256], f32, kind="ExternalInput")
    gidx_d = nc.dram_tensor("gidx", [128, NT * 4], mybir.dt.int32,
                            kind="ExternalInput")
    dstloc_d = nc.dram_tensor("dstloc", [128, NT], f32, kind="ExternalInput")
    ft_d = nc.dram_tensor("ft", [npc, NH * OD], f32, kind="ExternalOutput")

    with TileContext(nc) as tc:
        with (
            tc.tile_pool(name="const", bufs=1) as cpool,
            tc.tile_pool(name="edata", bufs=2) as edata_pool,
            tc.tile_pool(name="edataT", bufs=3) as edataT_pool,
            tc.tile_pool(name="zpool", bufs=3) as z_pool,
            tc.tile_pool(name="ohpool", bufs=3) as oh_pool,
            tc.tile_pool(name="small", bufs=4) as small_pool,
            tc.tile_pool(name="ftpool", bufs=2) as ft_pool,
            tc.tile_pool(name="psT", bufs=2, space="PSUM") as psT_pool,
            tc.tile_pool(name="psHA", bufs=2, space="PSUM") as psHA_pool,
            tc.tile_pool(name="psAGG", bufs=2, space="PSUM") as psAGG_pool,
        ):
            cm_sb = cpool.tile([128, 144], f32, tag="cm")
            nc.sync.dma_start(out=cm_sb[:], in_=cm_d[:, :])
            consts_sb = cpool.tile([128, 256], f32, tag="consts")
            nc.sync.dma_start(out=consts_sb[:], in_=consts_d[:, :])
            ident = consts_sb[:, 0:128]
            iota = consts_sb[:, 128:256]
            gidx_sb = cpool.tile([128, NT * 4], mybir.dt.int32, tag="gidx")
            nc.sync.dma_start(out=gidx_sb[:], in_=gidx_d[:, :])
            dstloc_sb = cpool.tile([128, NT], f32, tag="dstloc")
            nc.sync.dma_start(out=dstloc_sb[:], in_=dstloc_d[:, :])

            for b in range(nbuckets):
                edata = edata_pool.tile([128, T * 4 * OD], f32, tag="edata")
                # one row per partition per call: the multi-row-per-partition
                # dest form mis-lowers on HW (validated by probe), this form
                # matches the production tile_scatter_add usage.
                for cc in range(T * 4):
                    nc.gpsimd.indirect_dma_start(
                        out=edata[:, cc * OD:(cc + 1) * OD],
                        out_offset=None,
                        in_=feat_d[:, :],
                        in_offset=IndirectOffsetOnAxis(
                            ap=gidx_sb[:, b * T * 4 + cc: b * T * 4 + cc + 1],
                            axis=0),
                    )
                ps_agg = psAGG_pool.tile([128, 520], f32, tag="agg")

                for t in range(T):
                    ps_T = psT_pool.tile([128, 256], f32, tag="psT")
                    nc.tensor.transpose(
                        out=ps_T[:, 0:128],
                        in_=edata[:, t * 256: t * 256 + 128],
                        identity=ident,
                    )
                    nc.tensor.transpose(
                        out=ps_T[:, 128:256],
                        in_=edata[:, t * 256 + 128: t * 256 + 256],
                        identity=ident,
                    )
                    edataT = edataT_pool.tile([128, 256], f32, tag="edataT")
                    nc.scalar.copy(out=edataT[:], in_=ps_T[:])

                    ps_ha = psHA_pool.tile([128, 72], f32, tag="ha")
                    nc.tensor.matmul(out=ps_ha[:], lhsT=edataT[:, 0:128],
                                     rhs=cm_sb[:, 0:72], start=True, stop=False)
                    nc.tensor.matmul(out=ps_ha[:], lhsT=edataT[:, 128:256],
                                     rhs=cm_sb[:, 72:144], start=False, stop=True)

                    tmp8 = small_pool.tile([128, 8], f32, tag="tmp8")
                    nc.vector.tensor_scalar_mul(tmp8[:], ps_ha[:, 64:72], ALPHA)
                    a_sb = small_pool.tile([128, 8], f32, tag="a8")
                    nc.vector.tensor_max(a_sb[:], tmp8[:], ps_ha[:, 64:72])
                    p_sb = small_pool.tile([128, 8], f32, tag="p8")
                    nc.scalar.activation(p_sb[:], a_sb[:],
                                         mybir.ActivationFunctionType.Exp)

                    z = z_pool.tile([128, 520], f32, tag="z")
                    nc.vector.tensor_mul(
                        z[:, 0:512].rearrange("p (h d) -> p h d", h=NH),
                        p_sb[:].unsqueeze(2).to_broadcast([128, NH, OD]),
                        ps_ha[:, 0:64].unsqueeze(1).to_broadcast([128, NH, OD]),
                    )
                    nc.vector.tensor_copy(z[:, 512:520], p_sb[:])

                    oh = oh_pool.tile([128, 128], f32, tag="oh")
                    nc.vector.tensor_scalar(
                        oh[:], iota, dstloc_sb[:, b * T + t: b * T + t + 1],
                        None, mybir.AluOpType.is_equal)

                    nc.tensor.matmul(out=ps_agg[:, 0:512], lhsT=oh[:],
                                     rhs=z[:, 0:512],
                                     start=(t == 0), stop=(t == T - 1))
                    nc.tensor.matmul(out=ps_agg[:, 512:520], lhsT=oh[:],
                                     rhs=z[:, 512:520],
                                     start=(t == 0), stop=(t == T - 1))

                s_sb = small_pool.tile([128, 8], f32, tag="s8")
                nc.vector.tensor_scalar_add(s_sb[:], ps_agg[:, 512:520], EPS)
                srec = small_pool.tile([128, 8], f32, tag="sr8")
                nc.vector.reciprocal(srec[:], s_sb[:])
                ft_sb = ft_pool.tile([128, 512], f32, tag="ft")
                nc.vector.tensor_mul(
                    ft_sb[:].rearrange("p (h d) -> p h d", h=NH),
                    ps_agg[:, 0:512].rearrange("p (h d) -> p h d", h=NH),
                    srec[:].unsqueeze(2).to_broadcast([128, NH, OD]),
                )
                nc.sync.dma_start(out=ft_d[b * PT:(b + 1) * PT, :], in_=ft_sb[:])

    nc.compile()
    return nc


def _get_program(T):
    if T not in _PROGRAM_CACHE:
        _PROGRAM_CACHE[T] = _build(T)
    return _PROGRAM_CACHE[T]


def kernel(mpinstances, iftargets, input_node_features, rvec, attn1_w, attn2):
    mp = np.asarray(mpinstances)
    ift = np.asarray(iftargets)
    feat = np.ascontiguousarray(np.asarray(input_node_features, np.float32))

    cm = _linear_maps(rvec, attn1_w, attn2)                  # [256, 72]
    cm_in = np.ascontiguousarray(
        np.concatenate([cm[:128], cm[128:]], axis=1))        # [128, 144]
    consts = np.concatenate(
        [np.eye(128, dtype=np.float32),
         np.broadcast_to(np.arange(128, dtype=np.float32), (128, 128))],
        axis=1)                                              # [128, 256]
    consts = np.ascontiguousarray(consts)

    dst = mp[:, -1].astype(np.int64)
    counts = np.bincount(dst, minlength=N_CORES * NPC)
    T = max(5, int(np.ceil(counts.reshape(-1, PT).sum(axis=1).max() / PT)))

    nc = _get_program(T)

    in_maps = []
    for c in range(N_CORES):
        gidx, dstloc = _prep_core(mp, c, T)
        in_maps.append({
            "feat": feat,
            "cm": cm_in,
            "consts": consts,
            "gidx": gidx,
            "dstloc": dstloc,
        })

    res = run_bass_kernel_spmd(nc, in_maps, list(range(N_CORES)))
    ft = np.concatenate([res.results[c]["ft"] for c in range(N_CORES)], axis=0)
    ft = ft[:N].reshape(N, NH, OD)
    return ift[:, 0].copy(), ft
